# revision 1
# baseline (speedup 1.0000x reference)
"""Two-layer GAT on 8 Trainium2 NeuronCores.

Strategy (dst-sharded, per the sharding hint):
  Launch A (nodes sharded): h1 = x @ W1, per-node attention logits as1/ad1.
  Launch B (edges sharded by dst): per-edge gather of h1[src] (indirect DMA,
    128 rows/instruction), edge softmax numerator/denominator aggregated per
    dst-tile via a PE matmul with an iota-compare selection matrix, then
    layer-2 node transforms (h2 = relu(out1) @ W2, as2/ad2).
  Launch C: same edge phase for layer 2 -> final output.
  Host does index prep only: edge sort/padding, int64->int32, and gathering
  per-edge scalar logits from the per-node tables between launches.
"""
import sys
import types

import numpy as np

# ---------------------------------------------------------------------------
# Environment patches (walrus here accepts at most ONE sync-wait per
# instruction; Tile emits more). Register NTFF hook if available.
# ---------------------------------------------------------------------------
try:
    from antenv.axon_hooks import get_axon_ntff_profile_hook  # noqa: F401
except ImportError:
    try:
        import antenv
        _mod = types.ModuleType("antenv.axon_hooks")
        _hook_slot = [None]
        _mod.set_axon_ntff_profile_hook = lambda h: _hook_slot.__setitem__(0, h)
        _mod.get_axon_ntff_profile_hook = lambda: _hook_slot[0]
        sys.modules["antenv.axon_hooks"] = _mod
        antenv.axon_hooks = _mod
        try:
            from trn_agent_boot.trn_boot import _ntff_profile_via_ctypes
            _mod.set_axon_ntff_profile_hook(
                _ntff_profile_via_ctypes("/opt/axon/libaxon_pjrt.so"))
        except Exception:
            pass
    except ImportError:
        pass

import concourse.bass as bass
import concourse.mybir as mybir
import concourse.tile as tile_mod
from concourse.tile import TileContext
from concourse.masks import make_identity

ScopedClock = tile_mod.ScopedClock
F32 = mybir.dt.float32
I32 = mybir.dt.int32
AF = mybir.ActivationFunctionType
OP = mybir.AluOpType


def _patched_drain_and_barrier(self, tick_clock, wait_clock):
    nc = self.nc
    probe = nc.sync.nop(nofuse=True, hint="tail_wait_probe")
    wait_clock.add_sem_waits(probe.ins, ScopedClock({None: tick_clock.global_clock}))
    si = probe.ins.sync_info
    waits = list(si.on_wait) if si and si.on_wait else []
    if len(waits) > 1:
        si.on_wait = waits[:1]
        for w in waits[1:]:
            n2 = nc.sync.nop(nofuse=True, hint="tail_wait_extra")
            si2 = n2.ins.sync_info
            if si2 is None:
                n2.ins.sync_info = mybir.SyncInfo(on_wait=[w], on_update=[])
            else:
                si2.on_wait = [w]
    nc.sync.drain()
    nc.all_engine_barrier(sem_only=True)
    popped = nc._tile_sem_poison_stack.pop()
    assert popped is self._sem_poison
    nc.clear_and_free_semaphores(list(self.sems.allocated().values()))
    nc.all_engine_barrier(sem_only=True)


tile_mod.TileContext._drain_and_barrier = _patched_drain_and_barrier


def _split_multiwait(nc):
    for fn in nc.m.functions:
        for bb in fn.blocks:
            new_insts = []
            changed = False
            for inst in bb.instructions:
                si = getattr(inst, "sync_info", None)
                if si is not None and si.on_wait and len(si.on_wait) > 1:
                    waits = list(si.on_wait)
                    for w in waits[:-1]:
                        new_insts.append(mybir.InstNoOp(
                            name=nc.get_next_instruction_name(),
                            engine=inst.engine,
                            sync_info=mybir.SyncInfo(on_wait=[w], on_update=[]),
                            text_hint="split_wait", bass_nofuse=True))
                    si.on_wait = [waits[-1]]
                    changed = True
                new_insts.append(inst)
            if changed:
                bb.instructions[:] = new_insts


# ---------------------------------------------------------------------------
# Problem constants
# ---------------------------------------------------------------------------
N_NODES = 50000
N_EDGES = 800000
IN_CH = 256
HID = 32
H1 = 4
D1 = H1 * HID      # 128
OUT_CH = 64
NEG = 0.2
NCORES = 8
P = 128

LAST_PROFILE_NS = None


def _new_nc():
    return bass.Bass("TRN2", target_bir_lowering=False, debug=False,
                     num_devices=NCORES)


def _run(nc, in_maps, trace=False, label=""):
    import time
    from concourse.bass_utils import run_bass_kernel_spmd
    _split_multiwait(nc)
    t0 = time.time()
    print(f"[kernel] launch {label} starting", file=sys.stderr, flush=True)
    res = run_bass_kernel_spmd(nc, in_maps, core_ids=list(range(NCORES)),
                               trace=trace)
    print(f"[kernel] launch {label} done in {time.time()-t0:.0f}s",
          file=sys.stderr, flush=True)
    return res


# ---------------------------------------------------------------------------
# Launch A: node-sharded  h1 = x @ W1, asad1 = [h1 . a_src | h1 . a_dst]
# ---------------------------------------------------------------------------
def build_A(nsh):
    nc = _new_nc()
    x = nc.dram_tensor("x", [nsh, IN_CH], F32, kind="ExternalInput")
    w1 = nc.dram_tensor("w1", [IN_CH, D1], F32, kind="ExternalInput")
    a1m = nc.dram_tensor("a1m", [D1, 8], F32, kind="ExternalInput")
    h1o = nc.dram_tensor("h1o", [nsh, D1], F32, kind="ExternalOutput")
    asad1 = nc.dram_tensor("asad1", [8, nsh], F32, kind="ExternalOutput")
    nt = (nsh + P - 1) // P
    with TileContext(nc) as tc:
        with tc.tile_pool(name="const", bufs=1) as cp, \
             tc.tile_pool(name="sb", bufs=6) as sp, \
             tc.tile_pool(name="ps", bufs=2, space="PSUM") as pp:
            ident = cp.tile([P, P], F32)
            make_identity(nc, ident[:])
            w1a = cp.tile([P, D1], F32)
            nc.sync.dma_start(out=w1a[:], in_=w1[0:P, :])
            w1b = cp.tile([P, D1], F32)
            nc.sync.dma_start(out=w1b[:], in_=w1[P:IN_CH, :])
            a1 = cp.tile([D1, 8], F32)
            nc.sync.dma_start(out=a1[:], in_=a1m[:, :])
            for i in range(nt):
                ni = min(P, nsh - i * P)
                xt = sp.tile([P, IN_CH], F32, tag="xt")
                nc.sync.dma_start(out=xt[:ni, :], in_=x[i*P:i*P+ni, :])
                h1ps = pp.tile([P, P], F32, tag="h1T")
                for half in range(2):
                    xTps = pp.tile([P, P], F32, tag="xT")
                    nc.tensor.transpose(out=xTps[:, :ni],
                                        in_=xt[:ni, half*P:(half+1)*P],
                                        identity=ident[:ni, :ni])
                    xT = sp.tile([P, P], F32, tag="xTsb")
                    nc.vector.tensor_copy(out=xT[:, :ni], in_=xTps[:, :ni])
                    nc.tensor.matmul(out=h1ps[:, :ni],
                                     lhsT=(w1a if half == 0 else w1b)[:],
                                     rhs=xT[:, :ni],
                                     start=(half == 0), stop=(half == 1))
                h1T = sp.tile([P, P], F32, tag="h1Tsb")
                nc.vector.tensor_copy(out=h1T[:, :ni], in_=h1ps[:, :ni])
                aps = pp.tile([8, P], F32, tag="aps")
                nc.tensor.matmul(out=aps[:, :ni], lhsT=a1[:], rhs=h1T[:, :ni],
                                 start=True, stop=True)
                asb = sp.tile([8, P], F32, tag="asb")
                nc.vector.tensor_copy(out=asb[:, :ni], in_=aps[:, :ni])
                nc.scalar.dma_start(out=asad1[:, i*P:i*P+ni], in_=asb[:, :ni])
                h1p = pp.tile([P, P], F32, tag="h1row")
                nc.tensor.transpose(out=h1p[:ni, :], in_=h1T[:, :ni],
                                    identity=ident[:, :])
                h1sb = sp.tile([P, P], F32, tag="h1sb")
                nc.vector.tensor_copy(out=h1sb[:ni, :], in_=h1p[:ni, :])
                nc.scalar.dma_start(out=h1o[i*P:i*P+ni, :], in_=h1sb[:ni, :])
    return nc


# ---------------------------------------------------------------------------
# Launch B: edge phase of layer 1 + node transform of layer 2
# ---------------------------------------------------------------------------
def build_B(nslots, qs):
    T = sum(qs)
    npad = nslots * P
    nc = _new_nc()
    h1t = nc.dram_tensor("h1t", [N_NODES, D1], F32, kind="ExternalInput")
    idx = nc.dram_tensor("idx", [P, T], I32, kind="ExternalInput")
    dstloc = nc.dram_tensor("dstloc", [P, T], F32, kind="ExternalInput")
    asade = nc.dram_tensor("asade", [P, T * 8], F32, kind="ExternalInput")
    w2 = nc.dram_tensor("w2", [D1, OUT_CH], F32, kind="ExternalInput")
    a2m = nc.dram_tensor("a2m", [OUT_CH, 2], F32, kind="ExternalInput")
    b1r = nc.dram_tensor("b1r", [1, D1], F32, kind="ExternalInput")
    h2o = nc.dram_tensor("h2o", [npad, OUT_CH], F32, kind="ExternalOutput")
    asad2 = nc.dram_tensor("asad2", [2, npad], F32, kind="ExternalOutput")
    with TileContext(nc) as tc:
        with tc.tile_pool(name="const", bufs=1) as cp, \
             tc.tile_pool(name="g", bufs=16) as gp, \
             tc.tile_pool(name="gb", bufs=3) as gbp, \
             tc.tile_pool(name="small", bufs=6) as sp, \
             tc.tile_pool(name="fin", bufs=2) as fp, \
             tc.tile_pool(name="agg", bufs=4, space="PSUM") as aggp, \
             tc.tile_pool(name="finps", bufs=2, space="PSUM") as finp:
            ident = cp.tile([P, P], F32)
            make_identity(nc, ident[:])
            eps = cp.tile([P, 1], F32)
            nc.vector.memset(eps[:], 1e-16)
            neg_c = cp.tile([P, 1], F32)
            nc.vector.memset(neg_c[:], NEG)
            idx_sb = cp.tile([P, T], I32)
            nc.sync.dma_start(out=idx_sb[:], in_=idx[:, :])
            dl_sb = cp.tile([P, T], F32)
            nc.sync.dma_start(out=dl_sb[:], in_=dstloc[:, :])
            ae_sb = cp.tile([P, T, 8], F32)
            nc.sync.dma_start(out=ae_sb[:].rearrange("p a b -> p (a b)"),
                              in_=asade[:, :])
            EX = cp.tile([P, T, 4], F32)
            LR = cp.tile([P, T, 4], F32)
            nc.vector.tensor_add(out=LR[:], in0=ae_sb[:, :, 0:4],
                                 in1=ae_sb[:, :, 4:8])
            LRf = LR[:].rearrange("p a b -> p (a b)")
            EXf = EX[:].rearrange("p a b -> p (a b)")
            nc.vector.tensor_tensor(out=EXf, in0=LRf,
                                    in1=neg_c[:].to_broadcast([P, T * 4]),
                                    op=OP.mult)
            nc.vector.tensor_tensor(out=LRf, in0=LRf, in1=EXf, op=OP.max)
            nc.scalar.activation(out=EXf, in_=LRf, func=AF.Exp)
            w2sb = cp.tile([D1, OUT_CH], F32)
            nc.sync.dma_start(out=w2sb[:], in_=w2[:, :])
            a2sb = cp.tile([OUT_CH, 2], F32)
            nc.sync.dma_start(out=a2sb[:], in_=a2m[:, :])
            iota_i = cp.tile([P, P], I32)
            nc.gpsimd.iota(iota_i[:], [[1, P]], channel_multiplier=0)
            iota_f = cp.tile([P, P], F32)
            nc.vector.tensor_copy(out=iota_f[:], in_=iota_i[:])
            b1row = cp.tile([1, D1], F32)
            nc.sync.dma_start(out=b1row[:], in_=b1r[:, :])
            ones1 = cp.tile([1, P], F32)
            nc.vector.memset(ones1[:], 1.0)
            b1ps = finp.tile([P, D1], F32, tag="fin")
            nc.tensor.matmul(out=b1ps[:], lhsT=ones1[:], rhs=b1row[:],
                             start=True, stop=True)
            b1rep = cp.tile([P, D1], F32)
            nc.vector.tensor_copy(out=b1rep[:], in_=b1ps[:])

            t = 0
            for j, qj in enumerate(qs):
                ps = aggp.tile([P, D1 + 4], F32, tag="agg")
                gblock = gbp.tile([P, qj, D1], F32, tag="gb")
                for q in range(qj):
                    nc.gpsimd.indirect_dma_start(
                        out=gblock[:, q, :], out_offset=None, in_=h1t[:],
                        in_offset=bass.IndirectOffsetOnAxis(
                            ap=idx_sb[:, t+q:t+q+1], axis=0))
                for q in range(qj):
                    g = gblock[:, t + q - t, :] if False else gblock
                    ex = EX[:, t, :]
                    m = gp.tile([P, D1 + 4], F32, tag="m")
                    ex3 = bass.AP(ex.tensor, ex.offset,
                                  list(ex.ap) + [[0, HID]])
                    gq = gblock[:, q, :]
                    nc.vector.tensor_tensor(
                        out=m[:, 0:D1].rearrange("p (a b) -> p a b", a=H1),
                        in0=bass.AP(gq.tensor, gq.offset,
                                    [gq.ap[0], [HID, H1], [1, HID]]),
                        in1=ex3, op=OP.mult)
                    nc.vector.tensor_copy(out=m[:, D1:D1+4], in_=ex)
                    a01 = gp.tile([P, P], F32, tag="a01")
                    nc.vector.tensor_tensor(
                        out=a01[:], in0=dl_sb[:, t:t+1].to_broadcast([P, P]),
                        in1=iota_f[:], op=OP.is_equal)
                    nc.tensor.matmul(out=ps[:], lhsT=a01[:], rhs=m[:],
                                     start=(q == 0), stop=(q == qj - 1))
                    t += 1
                # finalize dst-tile j
                dr = sp.tile([P, 4], F32, tag="dr")
                nc.scalar.activation(out=dr[:], in_=ps[:, D1:D1+4],
                                     func=AF.Identity, bias=eps[:])
                nc.vector.reciprocal(out=dr[:], in_=dr[:])
                z = fp.tile([P, D1], F32, tag="z")
                for h in range(H1):
                    nc.vector.tensor_tensor(
                        out=z[:, h*HID:(h+1)*HID], in0=ps[:, h*HID:(h+1)*HID],
                        in1=dr[:, h:h+1].to_broadcast([P, HID]), op=OP.mult)
                nc.vector.tensor_add(out=z[:], in0=z[:], in1=b1rep[:])
                nc.scalar.activation(out=z[:], in_=z[:], func=AF.Relu)
                zTps = finp.tile([P, P], F32, tag="fin")
                nc.tensor.transpose(out=zTps[:], in_=z[:], identity=ident[:])
                zT = fp.tile([P, P], F32, tag="zT")
                nc.vector.tensor_copy(out=zT[:], in_=zTps[:])
                h2Tps = finp.tile([OUT_CH, P], F32, tag="fin")
                nc.tensor.matmul(out=h2Tps[:], lhsT=w2sb[:], rhs=zT[:],
                                 start=True, stop=True)
                h2T = fp.tile([OUT_CH, P], F32, tag="h2T")
                nc.vector.tensor_copy(out=h2T[:], in_=h2Tps[:])
                a2ps = finp.tile([2, P], F32, tag="fin")
                nc.tensor.matmul(out=a2ps[:], lhsT=a2sb[:], rhs=h2T[:],
                                 start=True, stop=True)
                a2sb_t = fp.tile([2, P], F32, tag="a2")
                nc.vector.tensor_copy(out=a2sb_t[:], in_=a2ps[:])
                nc.scalar.dma_start(out=asad2[:, j*P:(j+1)*P], in_=a2sb_t[:, :])
                h2ps = finp.tile([P, OUT_CH], F32, tag="fin")
                nc.tensor.transpose(out=h2ps[:, :], in_=h2T[:, :],
                                    identity=ident[:OUT_CH, :OUT_CH])
                h2sb = fp.tile([P, OUT_CH], F32, tag="h2")
                nc.vector.tensor_copy(out=h2sb[:, :], in_=h2ps[:, :])
                nc.scalar.dma_start(out=h2o[j*P:(j+1)*P, :], in_=h2sb[:, :])
    return nc


# ---------------------------------------------------------------------------
# Launch C: edge phase of layer 2 -> final output
# ---------------------------------------------------------------------------
def build_C(nslots, qs):
    T = sum(qs)
    npad = nslots * P
    nc = _new_nc()
    h2t = nc.dram_tensor("h2t", [N_NODES, OUT_CH], F32, kind="ExternalInput")
    idx = nc.dram_tensor("idx", [P, T], I32, kind="ExternalInput")
    dstloc = nc.dram_tensor("dstloc", [P, T], F32, kind="ExternalInput")
    asade = nc.dram_tensor("asade", [P, T * 2], F32, kind="ExternalInput")
    b2r = nc.dram_tensor("b2r", [1, OUT_CH], F32, kind="ExternalInput")
    outo = nc.dram_tensor("outo", [npad, OUT_CH], F32, kind="ExternalOutput")
    with TileContext(nc) as tc:
        with tc.tile_pool(name="const", bufs=1) as cp, \
             tc.tile_pool(name="g", bufs=16) as gp, \
             tc.tile_pool(name="gb", bufs=3) as gbp, \
             tc.tile_pool(name="small", bufs=6) as sp, \
             tc.tile_pool(name="fin", bufs=2) as fp, \
             tc.tile_pool(name="agg", bufs=4, space="PSUM") as aggp, \
             tc.tile_pool(name="finps", bufs=2, space="PSUM") as finp:
            eps = cp.tile([P, 1], F32)
            nc.vector.memset(eps[:], 1e-16)
            neg_c = cp.tile([P, 1], F32)
            nc.vector.memset(neg_c[:], NEG)
            idx_sb = cp.tile([P, T], I32)
            nc.sync.dma_start(out=idx_sb[:], in_=idx[:, :])
            dl_sb = cp.tile([P, T], F32)
            nc.sync.dma_start(out=dl_sb[:], in_=dstloc[:, :])
            ae_sb = cp.tile([P, T, 2], F32)
            nc.sync.dma_start(out=ae_sb[:].rearrange("p a b -> p (a b)"),
                              in_=asade[:, :])
            EX = cp.tile([P, T, 1], F32)
            LR = cp.tile([P, T, 1], F32)
            nc.vector.tensor_add(out=LR[:], in0=ae_sb[:, :, 0:1],
                                 in1=ae_sb[:, :, 1:2])
            LRf = LR[:].rearrange("p a b -> p (a b)")
            EXf = EX[:].rearrange("p a b -> p (a b)")
            nc.vector.tensor_tensor(out=EXf, in0=LRf,
                                    in1=neg_c[:].to_broadcast([P, T]),
                                    op=OP.mult)
            nc.vector.tensor_tensor(out=LRf, in0=LRf, in1=EXf, op=OP.max)
            nc.scalar.activation(out=EXf, in_=LRf, func=AF.Exp)
            iota_i = cp.tile([P, P], I32)
            nc.gpsimd.iota(iota_i[:], [[1, P]], channel_multiplier=0)
            iota_f = cp.tile([P, P], F32)
            nc.vector.tensor_copy(out=iota_f[:], in_=iota_i[:])
            b2row = cp.tile([1, OUT_CH], F32)
            nc.sync.dma_start(out=b2row[:], in_=b2r[:, :])
            ones1 = cp.tile([1, P], F32)
            nc.vector.memset(ones1[:], 1.0)
            b2ps = finp.tile([P, OUT_CH], F32, tag="fin")
            nc.tensor.matmul(out=b2ps[:], lhsT=ones1[:], rhs=b2row[:],
                             start=True, stop=True)
            b2rep = cp.tile([P, OUT_CH], F32)
            nc.vector.tensor_copy(out=b2rep[:], in_=b2ps[:])

            t = 0
            for j, qj in enumerate(qs):
                ps = aggp.tile([P, OUT_CH + 1], F32, tag="agg")
                gblock = gbp.tile([P, qj, OUT_CH], F32, tag="gb")
                for q in range(qj):
                    nc.gpsimd.indirect_dma_start(
                        out=gblock[:, q, :], out_offset=None, in_=h2t[:],
                        in_offset=bass.IndirectOffsetOnAxis(
                            ap=idx_sb[:, t+q:t+q+1], axis=0))
                for q in range(qj):
                    ex = EX[:, t, :]
                    m = gp.tile([P, OUT_CH + 1], F32, tag="m")
                    ex2 = bass.AP(ex.tensor, ex.offset,
                                  [ex.ap[0], [0, OUT_CH]])
                    nc.vector.tensor_tensor(out=m[:, 0:OUT_CH],
                                            in0=gblock[:, q, :],
                                            in1=ex2, op=OP.mult)
                    nc.vector.tensor_copy(out=m[:, OUT_CH:OUT_CH+1], in_=ex)
                    a01 = gp.tile([P, P], F32, tag="a01")
                    nc.vector.tensor_tensor(
                        out=a01[:], in0=dl_sb[:, t:t+1].to_broadcast([P, P]),
                        in1=iota_f[:], op=OP.is_equal)
                    nc.tensor.matmul(out=ps[:], lhsT=a01[:], rhs=m[:],
                                     start=(q == 0), stop=(q == qj - 1))
                    t += 1
                dr = sp.tile([P, 1], F32, tag="dr")
                nc.scalar.activation(out=dr[:], in_=ps[:, OUT_CH:OUT_CH+1],
                                     func=AF.Identity, bias=eps[:])
                nc.vector.reciprocal(out=dr[:], in_=dr[:])
                o = fp.tile([P, OUT_CH], F32, tag="o")
                nc.vector.tensor_tensor(
                    out=o[:], in0=ps[:, 0:OUT_CH],
                    in1=dr[:, 0:1].to_broadcast([P, OUT_CH]), op=OP.mult)
                nc.vector.tensor_add(out=o[:], in0=o[:], in1=b2rep[:])
                nc.scalar.dma_start(out=outo[j*P:(j+1)*P, :], in_=o[:, :])
    return nc


# ---------------------------------------------------------------------------
# Host-side orchestration
# ---------------------------------------------------------------------------
def _prep_edges(src, dst, n):
    """Balanced global dst-tile assignment. Returns per-core dicts, qs,
    slot map slot_tiles[c][j] = global dst-tile id (-1 = empty)."""
    ndt_g = (n + P - 1) // P
    tile_of = dst // P
    cnt = np.bincount(tile_of, minlength=ndt_g)
    order = np.argsort(-cnt, kind="stable")
    assign = [[] for _ in range(NCORES)]
    loads = np.zeros(NCORES, np.int64)
    for g in order:
        c = int(np.argmin(loads))
        assign[c].append(int(g))
        loads[c] += cnt[g]
    nslots = max(len(a) for a in assign)
    slot_tiles = [a + [-1] * (nslots - len(a)) for a in assign]
    # edges grouped per global tile, dst-sorted within
    eorder = np.argsort(tile_of, kind="stable")
    s_sorted, d_sorted = src[eorder], dst[eorder]
    bounds = np.searchsorted(tile_of[eorder], np.arange(ndt_g + 1))
    qs = [max(1, max(-(-int(cnt[slot_tiles[c][j]]) // P)
                     if slot_tiles[c][j] >= 0 else 1
                     for c in range(NCORES)))
          for j in range(nslots)]
    # qs must bound edges per tile, not node count: recompute from edge counts
    ecnt = np.bincount(tile_of, minlength=ndt_g)
    qs = [max(1, max(-(-int(ecnt[slot_tiles[c][j]]) // P)
                     if slot_tiles[c][j] >= 0 else 1
                     for c in range(NCORES)))
          for j in range(nslots)]
    T = sum(qs)
    per_core = []
    for c in range(NCORES):
        IDX = np.zeros((P, T), np.int32)
        DL = np.full((P, T), -1.0, np.float32)
        SRC = np.zeros((P, T), np.int64)
        DST = np.zeros((P, T), np.int64)
        VALID = np.zeros((P, T), bool)
        t = 0
        for j, qj in enumerate(qs):
            g = slot_tiles[c][j]
            if g >= 0:
                lo, hi = bounds[g], bounds[g + 1]
                s = s_sorted[lo:hi]
                dl = (d_sorted[lo:hi] - g * P).astype(np.int64)
                dfull = d_sorted[lo:hi]
            else:
                s = np.zeros(0, np.int64)
                dl = np.zeros(0, np.int64)
                dfull = np.zeros(0, np.int64)
            nE = len(s)
            pad = qj * P - nE
            sp_ = np.concatenate([s, np.zeros(pad, np.int64)])
            dlp = np.concatenate([dl, np.full(pad, -1, np.int64)])
            vp = np.concatenate([np.ones(nE, bool), np.zeros(pad, bool)])
            dfp = np.concatenate([dfull, np.zeros(pad, np.int64)])
            for q in range(qj):
                IDX[:, t] = sp_[q*P:(q+1)*P]
                DL[:, t] = dlp[q*P:(q+1)*P]
                SRC[:, t] = sp_[q*P:(q+1)*P]
                DST[:, t] = dfp[q*P:(q+1)*P]
                VALID[:, t] = vp[q*P:(q+1)*P]
                t += 1
        per_core.append(dict(IDX=IDX, DL=DL.astype(np.float32), SRC=SRC,
                             DST=DST, VALID=VALID))
    return per_core, qs, slot_tiles, nslots


def kernel(x, edge_index, W1, a_src1, a_dst1, b1, W2, a_src2, a_dst2, b2,
           profile=False):
    global LAST_PROFILE_NS
    x = np.asarray(x, np.float32)
    edge_index = np.asarray(edge_index)
    W1 = np.asarray(W1, np.float32)
    W2 = np.asarray(W2, np.float32)
    a_src1 = np.asarray(a_src1, np.float32)
    a_dst1 = np.asarray(a_dst1, np.float32)
    a_src2 = np.asarray(a_src2, np.float32)
    a_dst2 = np.asarray(a_dst2, np.float32)
    b1 = np.asarray(b1, np.float32)
    b2 = np.asarray(b2, np.float32)
    n = x.shape[0]
    nsh = n // NCORES
    src = edge_index[0].astype(np.int64)
    dst = edge_index[1].astype(np.int64)

    per_core, qs, slot_tiles, nslots = _prep_edges(src, dst, n)
    total_ns = 0

    # A1MAT: [D1, 8] block-diagonal per-head a_src / a_dst
    a1m = np.zeros((D1, 8), np.float32)
    for h in range(H1):
        a1m[h*HID:(h+1)*HID, h] = a_src1[h]
        a1m[h*HID:(h+1)*HID, 4 + h] = a_dst1[h]
    a2m = np.stack([a_src2[0], a_dst2[0]], axis=1).astype(np.float32)

    # ---- Launch A ----
    ncA = build_A(nsh)
    in_maps = [{"x": x[c*nsh:(c+1)*nsh], "w1": W1, "a1m": a1m}
               for c in range(NCORES)]
    resA = _run(ncA, in_maps, trace=profile, label="A")
    if profile:
        total_ns += resA.exec_time_ns or 0
    h1full = np.concatenate([resA.results[c]["h1o"] for c in range(NCORES)], 0)
    asad1 = np.concatenate([resA.results[c]["asad1"].T for c in range(NCORES)], 0)

    # ---- Launch B ----
    ncB = build_B(nslots, qs)
    in_maps = []
    for c in range(NCORES):
        pc = per_core[c]
        T = pc["IDX"].shape[1]
        ae = np.zeros((P, T, 8), np.float32)
        ae[..., 0:4] = asad1[pc["SRC"], 0:4]
        ae[..., 4:8] = asad1[pc["DST"], 4:8]
        ae[~pc["VALID"]] = 0.0
        in_maps.append({
            "h1t": h1full, "idx": pc["IDX"], "dstloc": pc["DL"],
            "asade": ae.reshape(P, T * 8), "w2": W2, "a2m": a2m,
            "b1r": b1.reshape(1, D1)})
    resB = _run(ncB, in_maps, trace=profile, label="B")
    if profile:
        total_ns += resB.exec_time_ns or 0
    h2full = np.zeros((n, OUT_CH), np.float32)
    asad2 = np.zeros((n, 2), np.float32)
    for c in range(NCORES):
        for j, g in enumerate(slot_tiles[c]):
            if g < 0:
                continue
            rows = min(P, n - g * P)
            h2full[g*P:g*P+rows] = resB.results[c]["h2o"][j*P:j*P+rows]
            asad2[g*P:g*P+rows] = resB.results[c]["asad2"][:, j*P:j*P+rows].T

    # ---- Launch C ----
    ncC = build_C(nslots, qs)
    in_maps = []
    for c in range(NCORES):
        pc = per_core[c]
        T = pc["IDX"].shape[1]
        ae = np.zeros((P, T, 2), np.float32)
        ae[..., 0] = asad2[pc["SRC"], 0]
        ae[..., 1] = asad2[pc["DST"], 1]
        ae[~pc["VALID"]] = 0.0
        in_maps.append({
            "h2t": h2full, "idx": pc["IDX"], "dstloc": pc["DL"],
            "asade": ae.reshape(P, T * 2), "b2r": b2.reshape(1, OUT_CH)})
    resC = _run(ncC, in_maps, trace=profile, label="C")
    if profile:
        total_ns += resC.exec_time_ns or 0
        LAST_PROFILE_NS = total_ns
    out = np.zeros((n, OUT_CH), np.float32)
    for c in range(NCORES):
        for j, g in enumerate(slot_tiles[c]):
            if g < 0:
                continue
            rows = min(P, n - g * P)
            out[g*P:g*P+rows] = resC.results[c]["outo"][j*P:j*P+rows]
    return out.astype(np.float32)



# revision 18
# speedup vs baseline: 1.0564x; 1.0564x over previous
"""Two-layer GAT on 8 Trainium2 NeuronCores.

Strategy (dst-sharded):
  Launch A (nodes sharded): h1T = W1^T xT (bf16), logits asad1; writes the
    bf16 row-major h1(+b1) gather table.
  Launch B (edges sharded by 64-node dst tile): batched dma_gather of h1[src]
    (one SWDGE instruction per ~12-slot group, int16 indices split at 32768),
    per-edge exp-weighting in bf16, per-dst-tile aggregation via one-hot
    matmuls into PSUM, then layer-2 node transforms (h2 / asad2).
  Launch C: same edge phase for layer 2 -> final output.
  Host does index prep, logit gathers between launches, and output stitching.
"""
import sys
import types

import numpy as np
import ml_dtypes

BF = ml_dtypes.bfloat16

# ---------------------------------------------------------------------------
# Environment patches (walrus here accepts at most ONE sync-wait per
# instruction; Tile emits more). Register NTFF hook if available.
# ---------------------------------------------------------------------------
try:
    from antenv.axon_hooks import get_axon_ntff_profile_hook  # noqa: F401
except ImportError:
    try:
        import antenv
        _mod = types.ModuleType("antenv.axon_hooks")
        _hook_slot = [None]
        _mod.set_axon_ntff_profile_hook = lambda h: _hook_slot.__setitem__(0, h)
        _mod.get_axon_ntff_profile_hook = lambda: _hook_slot[0]
        sys.modules["antenv.axon_hooks"] = _mod
        antenv.axon_hooks = _mod
        try:
            from trn_agent_boot.trn_boot import _ntff_profile_via_ctypes
            _mod.set_axon_ntff_profile_hook(
                _ntff_profile_via_ctypes("/opt/axon/libaxon_pjrt.so"))
        except Exception:
            pass
    except ImportError:
        pass

import concourse.bass as bass
import concourse.mybir as mybir
import concourse.tile as tile_mod
from concourse.tile import TileContext
from concourse import library_config

ScopedClock = tile_mod.ScopedClock
F32 = mybir.dt.float32
BF16 = mybir.dt.bfloat16
I16 = mybir.dt.int16
AF = mybir.ActivationFunctionType
OP = mybir.AluOpType


def _patched_drain_and_barrier(self, tick_clock, wait_clock):
    nc = self.nc
    probe = nc.sync.nop(nofuse=True, hint="tail_wait_probe")
    wait_clock.add_sem_waits(probe.ins, ScopedClock({None: tick_clock.global_clock}))
    si = probe.ins.sync_info
    waits = list(si.on_wait) if si and si.on_wait else []
    if len(waits) > 1:
        si.on_wait = waits[:1]
        for w in waits[1:]:
            n2 = nc.sync.nop(nofuse=True, hint="tail_wait_extra")
            si2 = n2.ins.sync_info
            if si2 is None:
                n2.ins.sync_info = mybir.SyncInfo(on_wait=[w], on_update=[])
            else:
                si2.on_wait = [w]
    nc.sync.drain()
    nc.all_engine_barrier(sem_only=True)
    popped = nc._tile_sem_poison_stack.pop()
    assert popped is self._sem_poison
    nc.clear_and_free_semaphores(list(self.sems.allocated().values()))
    nc.all_engine_barrier(sem_only=True)


_ORIG_DRAIN_AND_BARRIER = tile_mod.TileContext._drain_and_barrier
tile_mod.TileContext._drain_and_barrier = _patched_drain_and_barrier


def _split_multiwait(nc):
    for fn in nc.m.functions:
        for bb in fn.blocks:
            new_insts = []
            changed = False
            for inst in bb.instructions:
                si = getattr(inst, "sync_info", None)
                if si is not None and si.on_wait and len(si.on_wait) > 1:
                    waits = list(si.on_wait)
                    for w in waits[:-1]:
                        new_insts.append(mybir.InstNoOp(
                            name=nc.get_next_instruction_name(),
                            engine=inst.engine,
                            sync_info=mybir.SyncInfo(on_wait=[w], on_update=[]),
                            text_hint="split_wait", bass_nofuse=True))
                    si.on_wait = [waits[-1]]
                    changed = True
                new_insts.append(inst)
            if changed:
                bb.instructions[:] = new_insts


# ---------------------------------------------------------------------------
# Problem constants
# ---------------------------------------------------------------------------
N_NODES = 50000
N_EDGES = 800000
IN_CH = 256
HID = 32
H1 = 4
D1 = H1 * HID      # 128
OUT_CH = 64
NEG = 0.2
NCORES = 8
P = 128
DW = 64            # dst tile width
GRP = 12           # slots per gather group
SPLIT = 32768      # int16 index split

NPADA = 6400       # padded nodes per core in launch A (25 steps of 256)
GCH = 36           # subtiles per dma_gather (4608 idxs). single_packet=True
                   # caps at 64 descs/DMA engine (1024 idxs) and crashes the
                   # Q7 above that; single_packet=False chunks packets and
                   # was validated on HW up to 7680 idxs per instruction.

LAST_PROFILE_NS = None


def _new_nc():
    return bass.Bass("TRN2", target_bir_lowering=False, debug=False,
                     num_devices=NCORES)


def _run(nc, in_maps, trace=False, label=""):
    import time
    from concourse.bass_utils import run_bass_kernel_spmd
    from concourse.library_overlay import lower_extended_insts
    lower_extended_insts(nc)
    _split_multiwait(nc)
    t0 = time.time()
    print(f"[kernel] launch {label} starting", file=sys.stderr, flush=True)
    res = run_bass_kernel_spmd(nc, in_maps, core_ids=list(range(NCORES)),
                               trace=trace)
    print(f"[kernel] launch {label} done in {time.time()-t0:.0f}s",
          file=sys.stderr, flush=True)
    return res


def _ap(apobj, offset, dims):
    return bass.AP(apobj.tensor, offset, dims)


# ---------------------------------------------------------------------------
# Launch A: node-sharded  h1T = W1^T xT, logits, bf16 h1(+b1) table
# ---------------------------------------------------------------------------
def build_A():
    nc = _new_nc()
    nst = NPADA // 256                       # 25 steps
    xt = nc.dram_tensor("xt", [nst, IN_CH, 256], BF16, kind="ExternalInput")
    w1 = nc.dram_tensor("w1", [IN_CH, D1], BF16, kind="ExternalInput")
    a1m = nc.dram_tensor("a1m", [D1, 8], BF16, kind="ExternalInput")
    idb = nc.dram_tensor("idb", [P, P], BF16, kind="ExternalInput")
    b1c = nc.dram_tensor("b1c", [D1, 1], F32, kind="ExternalInput")
    h1o = nc.dram_tensor("h1o", [NPADA, D1], BF16, kind="ExternalOutput")
    asad1 = nc.dram_tensor("asad1", [8, NPADA], F32, kind="ExternalOutput")
    with TileContext(nc) as tc:
        with tc.tile_pool(name="const", bufs=1) as cp, \
             tc.tile_pool(name="sb", bufs=3) as sp, \
             tc.tile_pool(name="ps", bufs=2, space="PSUM") as pp, \
             tc.tile_pool(name="ps2", bufs=2, space="PSUM") as pp2:
            w1a = cp.tile([P, D1], BF16)
            nc.sync.dma_start(out=w1a[:], in_=w1[0:P, :])
            w1b = cp.tile([P, D1], BF16)
            nc.sync.dma_start(out=w1b[:], in_=w1[P:IN_CH, :])
            a1sb = cp.tile([D1, 8], BF16)
            nc.sync.dma_start(out=a1sb[:], in_=a1m[:, :])
            identb = cp.tile([P, P], BF16)
            nc.sync.dma_start(out=identb[:], in_=idb[:, :])
            b1col = cp.tile([D1, 1], F32)
            nc.sync.dma_start(out=b1col[:], in_=b1c[:, :])
            for i in range(nst):
                xsb = sp.tile([P, 2, 256], BF16, tag="xt")
                xv = xt[:, :, :]
                nc.sync.dma_start(
                    out=xsb[:],
                    in_=_ap(xv, i * IN_CH * 256,
                            [[256, P], [P * 256, 2], [1, 256]]))
                h1ps = pp.tile([P, 256], F32, tag="h1T")
                nc.tensor.matmul(out=h1ps[:], lhsT=w1a[:], rhs=xsb[:, 0, :],
                                 start=True, stop=False)
                nc.tensor.matmul(out=h1ps[:], lhsT=w1b[:], rhs=xsb[:, 1, :],
                                 start=False, stop=True)
                h1T = sp.tile([P, 256], BF16, tag="h1Tsb")
                nc.vector.tensor_copy(out=h1T[:], in_=h1ps[:])
                aps = pp2.tile([8, 256], F32, tag="aps")
                nc.tensor.matmul(out=aps[:], lhsT=a1sb[:], rhs=h1T[:],
                                 start=True, stop=True)
                asb = sp.tile([8, 256], F32, tag="asb")
                nc.scalar.activation(out=asb[:], in_=aps[:], func=AF.Identity)
                nc.sync.dma_start(out=asad1[:, i*256:(i+1)*256], in_=asb[:])
                h1Tb = sp.tile([P, 256], BF16, tag="h1Tb")
                nc.scalar.activation(out=h1Tb[:], in_=h1ps[:],
                                     func=AF.Identity, bias=b1col[:])
                h1p = pp2.tile([P, 256], BF16, tag="h1row")
                for h in range(2):
                    nc.tensor.transpose(out=h1p[:, h*P:(h+1)*P],
                                        in_=h1Tb[:, h*P:(h+1)*P],
                                        identity=identb[:])
                h1sb = sp.tile([P, 256], BF16, tag="h1sb")
                nc.vector.tensor_copy(out=h1sb[:], in_=h1p[:])
                ov = h1o[:, :]
                nc.scalar.dma_start(
                    out=_ap(ov, i * 256 * D1, [[D1, P], [P * D1, 2], [1, D1]]),
                    in_=h1sb[:].rearrange("p (h d) -> p h d", h=2))
    return nc


# ---------------------------------------------------------------------------
# Launch B: edge phase of layer 1 + node transform of layer 2
# ---------------------------------------------------------------------------
def build_B(T, qgmax, groups, npad):
    """groups: list of dicts {tg, qlo, qhi, slots: [(j, lopos, hipos), ...]}
    lopos/hipos are positions local to the group buffer."""
    nc = _new_nc()
    h1t = nc.dram_tensor("h1t", [N_NODES, D1], BF16, kind="ExternalInput")
    idx = nc.dram_tensor("idx", [P, T * 8], I16, kind="ExternalInput")
    dl = nc.dram_tensor("dl", [P, T], BF16, kind="ExternalInput")
    lr = nc.dram_tensor("lr", [P, T * 4], BF16, kind="ExternalInput")
    w2 = nc.dram_tensor("w2", [D1, OUT_CH], BF16, kind="ExternalInput")
    wa2 = nc.dram_tensor("wa2", [D1, 2], BF16, kind="ExternalInput")
    iot = nc.dram_tensor("iot", [P, DW], BF16, kind="ExternalInput")
    idb = nc.dram_tensor("idb", [P, P], BF16, kind="ExternalInput")
    out1 = nc.dram_tensor("out1", [npad, 66], F32, kind="ExternalOutput")
    with TileContext(nc) as tc:
        with tc.tile_pool(name="const", bufs=1) as cp, \
             tc.tile_pool(name="gb", bufs=2) as gbp, \
             tc.tile_pool(name="a01p", bufs=2) as ap01, \
             tc.tile_pool(name="small", bufs=4) as sp, \
             tc.tile_pool(name="fin", bufs=3) as fp, \
             tc.tile_pool(name="agg", bufs=4, space="PSUM") as aggp, \
             tc.tile_pool(name="finps", bufs=2, space="PSUM") as finp:
            nc.gpsimd.load_library(library_config.mlp)
            idx_sb = cp.tile([P, T * 8], I16)
            nc.sync.dma_start(out=idx_sb[:], in_=idx[:, :])
            dl_sb = cp.tile([P, T], BF16)
            nc.sync.dma_start(out=dl_sb[:], in_=dl[:, :])
            iota = cp.tile([P, DW], BF16)
            nc.sync.dma_start(out=iota[:], in_=iot[:, :])
            identb = cp.tile([P, P], BF16)
            nc.sync.dma_start(out=identb[:], in_=idb[:, :])
            w2sb = cp.tile([D1, OUT_CH], BF16)
            nc.sync.dma_start(out=w2sb[:], in_=w2[:, :])
            wa2sb = cp.tile([D1, 2], BF16)
            nc.sync.dma_start(out=wa2sb[:], in_=wa2[:, :])
            neg_c = cp.tile([P, 1], BF16)
            nc.vector.memset(neg_c[:], NEG)
            eps = cp.tile([DW, 1], F32)
            nc.vector.memset(eps[:], 1e-16)
            lr_sb = cp.tile([P, T, 4], BF16)
            nc.sync.dma_start(out=lr_sb[:].rearrange("p a b -> p (a b)"),
                              in_=lr[:, :])
            EX = cp.tile([P, T, 4], BF16)
            LRf = lr_sb[:].rearrange("p a b -> p (a b)")
            EXf = EX[:].rearrange("p a b -> p (a b)")
            nc.vector.tensor_tensor(out=EXf, in0=LRf,
                                    in1=neg_c[:].to_broadcast([P, T * 4]),
                                    op=OP.mult)
            nc.vector.tensor_tensor(out=EXf, in0=LRf, in1=EXf, op=OP.max)
            nc.scalar.activation(out=EXf, in_=EXf, func=AF.Exp)

            nregs = {}
            for gm in groups:
                qlo, qhi, tg = gm["qlo"], gm["qhi"], gm["tg"]
                qg = qlo + qhi
                gb = gbp.tile([P, qgmax, D1], BF16, tag="gb")
                for (q0, qn, tab) in ((0, qlo, h1t[0:SPLIT, :]),
                                      (qlo, qhi, h1t[SPLIT:N_NODES, :])):
                    for c0 in range(0, qn, GCH):
                        cn = min(GCH, qn - c0)
                        if cn not in nregs:
                            nregs[cn] = nc.gpsimd.to_reg(cn * P)
                        nc.gpsimd.dma_gather(
                            gb[:, q0 + c0:q0 + c0 + cn, :], tab,
                            idx_sb[:, (tg + q0 + c0) * 8:
                                   (tg + q0 + c0 + cn) * 8],
                            cn * P, nregs[cn], D1, single_packet=False)
                gv = gb[:]
                g4 = _ap(gv, gv.offset,
                         [gv.ap[0], [D1, qg], [HID, H1], [1, HID]])
                ev = EX[:, tg:tg + qg, :]
                e4 = _ap(ev, ev.offset,
                         [ev.ap[0], [4, qg], [1, 4], [0, HID]])
                nc.vector.tensor_tensor(out=g4, in0=g4, in1=e4, op=OP.mult)
                a01 = ap01.tile([P, qgmax, DW], BF16, tag="a01")
                dv = dl_sb[:, tg:tg + qg]
                d3 = _ap(dv, dv.offset, [dv.ap[0], [1, qg], [0, DW]])
                iv = iota[:]
                i3 = _ap(iv, iv.offset, [iv.ap[0], [0, qg], [1, DW]])
                nc.vector.tensor_tensor(out=a01[:, 0:qg, :], in0=d3, in1=i3,
                                        op=OP.is_equal)
                for (j, lopos, hipos) in gm["slots"]:
                    ps = aggp.tile([DW, D1 + 4], F32, tag="agg")
                    poss = list(lopos) + list(hipos)
                    last = len(poss) - 1
                    for k, pos in enumerate(poss):
                        nc.tensor.matmul(out=ps[:, 0:D1],
                                         lhsT=a01[:, pos, :],
                                         rhs=gb[:, pos, :],
                                         start=(k == 0), stop=(k == last))
                    for k, pos in enumerate(poss):
                        nc.tensor.matmul(out=ps[:, D1:D1 + 4],
                                         lhsT=a01[:, pos, :],
                                         rhs=EX[:, tg + pos, :],
                                         start=(k == 0), stop=(k == last))
                    dr = sp.tile([DW, 4], F32, tag="dr")
                    nc.scalar.activation(out=dr[:], in_=ps[:, D1:D1 + 4],
                                         func=AF.Identity, bias=eps[:])
                    nc.vector.reciprocal(out=dr[:], in_=dr[:])
                    z = fp.tile([DW, D1], BF16, tag="z")
                    zv = z[:]
                    z3 = _ap(zv, zv.offset, [zv.ap[0], [HID, H1], [1, HID]])
                    pv = ps[:, 0:D1]
                    p3 = _ap(pv, pv.offset, [pv.ap[0], [HID, H1], [1, HID]])
                    dv2 = dr[:]
                    d3b = _ap(dv2, dv2.offset, [dv2.ap[0], [1, 4], [0, HID]])
                    nc.vector.tensor_tensor(out=z3, in0=p3, in1=d3b,
                                            op=OP.mult)
                    ztps = finp.tile([D1, DW], BF16, tag="zt")
                    nc.tensor.transpose(out=ztps[:], in_=z[:],
                                        identity=identb[0:DW, 0:DW])
                    zt = fp.tile([D1, DW], BF16, tag="ztsb")
                    nc.scalar.activation(out=zt[:], in_=ztps[:], func=AF.Relu)
                    ha = finp.tile([DW, 66], F32, tag="ha")
                    nc.tensor.matmul(out=ha[:, 0:OUT_CH], lhsT=zt[:],
                                     rhs=w2sb[:], start=True, stop=True)
                    nc.tensor.matmul(out=ha[:, OUT_CH:66], lhsT=zt[:],
                                     rhs=wa2sb[:], start=True, stop=True)
                    hasb = fp.tile([DW, 66], F32, tag="hasb")
                    nc.scalar.activation(out=hasb[:], in_=ha[:],
                                         func=AF.Identity)
                    nc.sync.dma_start(out=out1[j*DW:(j+1)*DW, :], in_=hasb[:])
    return nc


# ---------------------------------------------------------------------------
# Launch C: edge phase of layer 2 -> final output
# ---------------------------------------------------------------------------
def build_C(T, qgmax, groups, npad):
    nc = _new_nc()
    h2t = nc.dram_tensor("h2t", [N_NODES, OUT_CH], F32, kind="ExternalInput")
    idx = nc.dram_tensor("idx", [P, T * 8], I16, kind="ExternalInput")
    dl = nc.dram_tensor("dl", [P, T], BF16, kind="ExternalInput")
    lr = nc.dram_tensor("lr", [P, T], BF16, kind="ExternalInput")
    iot = nc.dram_tensor("iot", [P, DW], BF16, kind="ExternalInput")
    b2r = nc.dram_tensor("b2r", [1, OUT_CH], F32, kind="ExternalInput")
    outo = nc.dram_tensor("outo", [npad, OUT_CH], F32, kind="ExternalOutput")
    with TileContext(nc) as tc:
        with tc.tile_pool(name="const", bufs=1) as cp, \
             tc.tile_pool(name="gb", bufs=2) as gbp, \
             tc.tile_pool(name="mp", bufs=2) as mp, \
             tc.tile_pool(name="a01p", bufs=2) as ap01, \
             tc.tile_pool(name="small", bufs=4) as sp, \
             tc.tile_pool(name="fin", bufs=3) as fp, \
             tc.tile_pool(name="agg", bufs=4, space="PSUM") as aggp, \
             tc.tile_pool(name="finps", bufs=2, space="PSUM") as finp:
            nc.gpsimd.load_library(library_config.mlp)
            idx_sb = cp.tile([P, T * 8], I16)
            nc.sync.dma_start(out=idx_sb[:], in_=idx[:, :])
            dl_sb = cp.tile([P, T], BF16)
            nc.sync.dma_start(out=dl_sb[:], in_=dl[:, :])
            iota = cp.tile([P, DW], BF16)
            nc.sync.dma_start(out=iota[:], in_=iot[:, :])
            neg_c = cp.tile([P, 1], BF16)
            nc.vector.memset(neg_c[:], NEG)
            eps = cp.tile([DW, 1], F32)
            nc.vector.memset(eps[:], 1e-16)
            lr_sb = cp.tile([P, T], BF16)
            nc.sync.dma_start(out=lr_sb[:], in_=lr[:, :])
            EX = cp.tile([P, T], BF16)
            nc.vector.tensor_tensor(out=EX[:], in0=lr_sb[:],
                                    in1=neg_c[:].to_broadcast([P, T]),
                                    op=OP.mult)
            nc.vector.tensor_tensor(out=EX[:], in0=lr_sb[:], in1=EX[:],
                                    op=OP.max)
            nc.scalar.activation(out=EX[:], in_=EX[:], func=AF.Exp)
            b2row = cp.tile([1, OUT_CH], F32)
            nc.sync.dma_start(out=b2row[:], in_=b2r[:, :])
            ones1 = cp.tile([1, DW], F32)
            nc.vector.memset(ones1[:], 1.0)
            b2ps = finp.tile([DW, OUT_CH], F32, tag="zt")
            nc.tensor.matmul(out=b2ps[:], lhsT=ones1[:], rhs=b2row[:],
                             start=True, stop=True)
            b2rep = cp.tile([DW, OUT_CH], F32)
            nc.vector.tensor_copy(out=b2rep[:], in_=b2ps[:])

            nregs = {}
            for gm in groups:
                qlo, qhi, tg = gm["qlo"], gm["qhi"], gm["tg"]
                qg = qlo + qhi
                gbf = gbp.tile([P, qgmax, OUT_CH], F32, tag="gb")
                for (q0, qn, tab) in ((0, qlo, h2t[0:SPLIT, :]),
                                      (qlo, qhi, h2t[SPLIT:N_NODES, :])):
                    for c0 in range(0, qn, GCH):
                        cn = min(GCH, qn - c0)
                        if cn not in nregs:
                            nregs[cn] = nc.gpsimd.to_reg(cn * P)
                        nc.gpsimd.dma_gather(
                            gbf[:, q0 + c0:q0 + c0 + cn, :], tab,
                            idx_sb[:, (tg + q0 + c0) * 8:
                                   (tg + q0 + c0 + cn) * 8],
                            cn * P, nregs[cn], OUT_CH, single_packet=False)
                m = mp.tile([P, qgmax, OUT_CH], BF16, tag="m")
                gv = gbf[:, 0:qg, :]
                mv = m[:, 0:qg, :]
                ev = EX[:, tg:tg + qg]
                e3 = _ap(ev, ev.offset, [ev.ap[0], [1, qg], [0, OUT_CH]])
                nc.vector.tensor_tensor(out=mv, in0=gv, in1=e3, op=OP.mult)
                a01 = ap01.tile([P, qgmax, DW], BF16, tag="a01")
                dv = dl_sb[:, tg:tg + qg]
                d3 = _ap(dv, dv.offset, [dv.ap[0], [1, qg], [0, DW]])
                iv = iota[:]
                i3 = _ap(iv, iv.offset, [iv.ap[0], [0, qg], [1, DW]])
                nc.vector.tensor_tensor(out=a01[:, 0:qg, :], in0=d3, in1=i3,
                                        op=OP.is_equal)
                for (j, lopos, hipos) in gm["slots"]:
                    ps = aggp.tile([DW, OUT_CH + 1], F32, tag="agg")
                    poss = list(lopos) + list(hipos)
                    last = len(poss) - 1
                    for k, pos in enumerate(poss):
                        nc.tensor.matmul(out=ps[:, 0:OUT_CH],
                                         lhsT=a01[:, pos, :],
                                         rhs=m[:, pos, :],
                                         start=(k == 0), stop=(k == last))
                    for k, pos in enumerate(poss):
                        nc.tensor.matmul(out=ps[:, OUT_CH:OUT_CH + 1],
                                         lhsT=a01[:, pos, :],
                                         rhs=EX[:, tg + pos:tg + pos + 1],
                                         start=(k == 0), stop=(k == last))
                    dr = sp.tile([DW, 1], F32, tag="dr")
                    nc.scalar.activation(out=dr[:], in_=ps[:, OUT_CH:OUT_CH+1],
                                         func=AF.Identity, bias=eps[:])
                    nc.vector.reciprocal(out=dr[:], in_=dr[:])
                    o = fp.tile([DW, OUT_CH], F32, tag="o")
                    nc.vector.tensor_tensor(
                        out=o[:], in0=ps[:, 0:OUT_CH],
                        in1=dr[:].to_broadcast([DW, OUT_CH]), op=OP.mult)
                    nc.vector.tensor_add(out=o[:], in0=o[:], in1=b2rep[:])
                    nc.sync.dma_start(out=outo[j*DW:(j+1)*DW, :], in_=o[:])
    return nc


# ---------------------------------------------------------------------------
# Host-side edge prep
# ---------------------------------------------------------------------------
def _prep_edges(src, dst, n):
    ndt = (n + DW - 1) // DW
    tile_of = dst // DW
    ecnt = np.bincount(tile_of, minlength=ndt)
    order = np.argsort(-ecnt, kind="stable")
    assign = [[] for _ in range(NCORES)]
    loads = np.zeros(NCORES, np.int64)
    for g in order:
        c = int(np.argmin(loads))
        assign[c].append(int(g))
        loads[c] += ecnt[g]
    nslots = max(len(a) for a in assign)
    slot_tiles = [a + [-1] * (nslots - len(a)) for a in assign]

    eorder = np.argsort(tile_of, kind="stable")
    s_sorted, d_sorted = src[eorder], dst[eorder]
    bounds = np.searchsorted(tile_of[eorder], np.arange(ndt + 1))

    # per (c, j) lo/hi edge lists (sorted by src)
    edges = [[None] * nslots for _ in range(NCORES)]
    nlo = np.zeros((NCORES, nslots), np.int64)
    nhi = np.zeros((NCORES, nslots), np.int64)
    for c in range(NCORES):
        for j in range(nslots):
            g = slot_tiles[c][j]
            if g < 0:
                lo = (np.zeros(0, np.int64), np.zeros(0, np.int64))
                hi = (np.zeros(0, np.int64), np.zeros(0, np.int64))
            else:
                a, b = bounds[g], bounds[g + 1]
                s = s_sorted[a:b]
                d = d_sorted[a:b] - g * DW
                mlo = s < SPLIT
                slo, dlo = s[mlo], d[mlo]
                shi, dhi = s[~mlo] - SPLIT, d[~mlo]
                olo = np.argsort(slo, kind="stable")
                ohi = np.argsort(shi, kind="stable")
                lo = (slo[olo], dlo[olo])
                hi = (shi[ohi], dhi[ohi])
            edges[c][j] = (lo, hi)
            nlo[c][j] = len(lo[0])
            nhi[c][j] = len(hi[0])
    qlo = np.maximum(1, -(-nlo.max(axis=0) // P))
    qhi = np.maximum(1, -(-nhi.max(axis=0) // P))

    # groups of GRP slots; columns: [lo subtiles slot-major][hi subtiles]
    groups = []
    tg = 0
    for j0 in range(0, nslots, GRP):
        jl = list(range(j0, min(nslots, j0 + GRP)))
        qlo_g = int(qlo[jl].sum())
        qhi_g = int(qhi[jl].sum())
        slots = []
        lo_off, hi_off = 0, qlo_g
        for j in jl:
            lop = list(range(lo_off, lo_off + int(qlo[j])))
            hip = list(range(hi_off, hi_off + int(qhi[j])))
            slots.append((j, lop, hip))
            lo_off += int(qlo[j])
            hi_off += int(qhi[j])
        groups.append({"tg": tg, "qlo": qlo_g, "qhi": qhi_g, "slots": slots})
        tg += qlo_g + qhi_g
    T = tg
    qgmax = max(g["qlo"] + g["qhi"] for g in groups)

    # per-core host arrays
    per_core = []
    for c in range(NCORES):
        IDX16 = np.zeros((16, T * 8), np.int16)
        DLv = np.full((P, T), -1.0, np.float32)
        SRC = np.zeros((P, T), np.int64)
        DST = np.zeros((P, T), np.int64)
        VALID = np.zeros((P, T), bool)
        for gm in groups:
            for (j, lop, hip) in gm["slots"]:
                g = slot_tiles[c][j]
                for (part, positions) in (("lo", lop), ("hi", hip)):
                    s, d = edges[c][j][0 if part == "lo" else 1]
                    nE = len(s)
                    cap = len(positions) * P
                    pad = cap - nE
                    sp_ = np.concatenate([s, np.zeros(pad, np.int64)])
                    dlp = np.concatenate([d, np.full(pad, -1, np.int64)])
                    vp = np.concatenate([np.ones(nE, bool),
                                         np.zeros(pad, bool)])
                    base = 0 if part == "lo" else SPLIT
                    for k, pos in enumerate(positions):
                        t = gm["tg"] + pos
                        se = sp_[k*P:(k+1)*P]
                        de = dlp[k*P:(k+1)*P]
                        ve = vp[k*P:(k+1)*P]
                        r = np.arange(P)
                        IDX16[r % 16, t * 8 + r // 16] = se.astype(np.int16)
                        DLv[:, t] = de
                        SRC[:, t] = se + base
                        DST[:, t] = np.where(ve, g * DW + de, 0)
                        VALID[:, t] = ve
        IDXr = np.tile(IDX16, (8, 1))
        per_core.append(dict(IDX=IDXr, DL=DLv.astype(BF), SRC=SRC, DST=DST,
                             VALID=VALID))
    return per_core, groups, slot_tiles, nslots, T, qgmax


def kernel(x, edge_index, W1, a_src1, a_dst1, b1, W2, a_src2, a_dst2, b2,
           profile=False):
    global LAST_PROFILE_NS
    x = np.asarray(x, np.float32)
    edge_index = np.asarray(edge_index)
    W1 = np.asarray(W1, np.float32)
    W2 = np.asarray(W2, np.float32)
    a_src1 = np.asarray(a_src1, np.float32)
    a_dst1 = np.asarray(a_dst1, np.float32)
    a_src2 = np.asarray(a_src2, np.float32)
    a_dst2 = np.asarray(a_dst2, np.float32)
    b1 = np.asarray(b1, np.float32)
    b2 = np.asarray(b2, np.float32)
    n = x.shape[0]
    src = edge_index[0].astype(np.int64)
    dst = edge_index[1].astype(np.int64)

    per_core, groups, slot_tiles, nslots, T, qgmax = _prep_edges(src, dst, n)
    npad = nslots * DW
    total_ns = 0

    # identity / iota constants
    idb = np.eye(P, dtype=BF)
    iot = np.tile(np.arange(DW, dtype=np.float32)[None, :].astype(BF),
                  (P, 1))

    # A1MAT: [D1, 8] block-diagonal per-head a_src / a_dst
    a1m = np.zeros((D1, 8), np.float32)
    for h in range(H1):
        a1m[h*HID:(h+1)*HID, h] = a_src1[h]
        a1m[h*HID:(h+1)*HID, 4 + h] = a_dst1[h]

    # ---- Launch A ----
    nst = NPADA // 256
    xpad = np.zeros((NCORES * NPADA, IN_CH), np.float32)
    xpad[:n] = x
    xt_all = xpad.reshape(NCORES, nst, 256, IN_CH).transpose(0, 1, 3, 2)
    xt_all = np.ascontiguousarray(xt_all).astype(BF)
    ncA = build_A()
    in_maps = [{"xt": xt_all[c], "w1": W1.astype(BF), "a1m": a1m.astype(BF),
                "idb": idb, "b1c": b1.reshape(D1, 1)}
               for c in range(NCORES)]
    resA = _run(ncA, in_maps, trace=profile, label="A")
    if profile:
        total_ns += resA.exec_time_ns or 0
    h1full = np.concatenate([np.asarray(resA.results[c]["h1o"])
                             for c in range(NCORES)], 0)[:n]
    asad1 = np.concatenate([np.asarray(resA.results[c]["asad1"]).T
                            for c in range(NCORES)], 0)[:n]

    # ---- Launch B ----
    wa2 = (W2 @ np.stack([a_src2[0], a_dst2[0]], axis=1)).astype(BF)
    ncB = build_B(T, qgmax, groups, npad)
    in_maps = []
    for c in range(NCORES):
        pc = per_core[c]
        lr1 = asad1[pc["SRC"], 0:4] + asad1[pc["DST"], 4:8]
        lr1[~pc["VALID"]] = 0.0
        in_maps.append({
            "h1t": h1full, "idx": pc["IDX"], "dl": pc["DL"],
            "lr": lr1.astype(BF).reshape(P, T * 4),
            "w2": W2.astype(BF), "wa2": wa2,
            "iot": iot, "idb": idb})
    resB = _run(ncB, in_maps, trace=profile, label="B")
    if profile:
        total_ns += resB.exec_time_ns or 0
    h2full = np.zeros((n, OUT_CH), np.float32)
    asad2 = np.zeros((n, 2), np.float32)
    for c in range(NCORES):
        o1 = np.asarray(resB.results[c]["out1"])
        for j, g in enumerate(slot_tiles[c]):
            if g < 0:
                continue
            rows = min(DW, n - g * DW)
            h2full[g*DW:g*DW+rows] = o1[j*DW:j*DW+rows, 0:OUT_CH]
            asad2[g*DW:g*DW+rows] = o1[j*DW:j*DW+rows, OUT_CH:66]

    # ---- Launch C ----
    ncC = build_C(T, qgmax, groups, npad)
    in_maps = []
    for c in range(NCORES):
        pc = per_core[c]
        lr2 = asad2[pc["SRC"], 0] + asad2[pc["DST"], 1]
        lr2[~pc["VALID"]] = 0.0
        in_maps.append({
            "h2t": h2full, "idx": pc["IDX"], "dl": pc["DL"],
            "lr": lr2.astype(BF), "iot": iot,
            "b2r": b2.reshape(1, OUT_CH)})
    resC = _run(ncC, in_maps, trace=profile, label="C")
    if profile:
        total_ns += resC.exec_time_ns or 0
        LAST_PROFILE_NS = total_ns
    out = np.zeros((n, OUT_CH), np.float32)
    for c in range(NCORES):
        oc = np.asarray(resC.results[c]["outo"])
        for j, g in enumerate(slot_tiles[c]):
            if g < 0:
                continue
            rows = min(DW, n - g * DW)
            out[g*DW:g*DW+rows] = oc[j*DW:j*DW+rows]
    return out.astype(np.float32)
